# revision 6
# baseline (speedup 1.0000x reference)
"""Trainium2 Bass kernel for nn_CellAnnotator (per-pixel 8x8 locally-connected
weighted pooling with normalization), SPMD across 8 NeuronCores.

Contract: kernel(**inputs) takes FULL inputs (x0 [512,512,128] f32,
weights [512,512,64] f32, cnts [512,512,1] f32) and returns the FULL
output [512,512,128] f32.

Sharding: rows (H) split across 8 cores, 64 output rows each; each core's
input shard carries a 3+4-row halo (built host-side, zero-padded at the
image borders), so no device-to-device communication is needed.
"""

import numpy as np
from contextlib import ExitStack

import concourse.bass as bass
import concourse.bacc as bacc
import concourse.mybir as mybir
import concourse.tile as tile
from concourse.ap import AP
from concourse.bass_utils import run_bass_kernel_spmd

# Problem constants (hardcoded per contract)
H, W, C = 512, 512, 128
ROI = 8
TAPS = ROI * ROI
PAD_LO, PAD_HI = 3, 4          # XLA SAME padding for even kernel
NCORES = 8
ROWS = H // NCORES             # 64 output rows per core
IN_ROWS = ROWS + ROI - 1       # 71 input rows (halo included)
WPAD = W + ROI                 # padded width: cols -3 .. 516 (520)
CCH = C + 1                    # x channels + cnts as channel 128

# Column blocking: 128-wide input-col tiles serve up to 121 output pixels
BLK = 121
BLKS = [(b * BLK, min(BLK, W - b * BLK)) for b in range((W + BLK - 1) // BLK)]

_CACHE = {}


def _build_nc(rep=1):
    f32 = mybir.dt.float32
    nc = bacc.Bacc("TRN2", target_bir_lowering=False, debug=False,
                   num_devices=NCORES)
    xc = nc.dram_tensor("xc", [IN_ROWS, WPAD, CCH], f32, kind="ExternalInput")
    wt = nc.dram_tensor("wt", [ROWS, W, TAPS], f32, kind="ExternalInput")
    out = nc.dram_tensor("out", [ROWS, W, C], f32, kind="ExternalOutput")

    with tile.TileContext(nc) as tc:
        with ExitStack() as ctx:
            if rep > 1:
                rv = ctx.enter_context(tc.For_i(0, rep, 1))
            xpool = ctx.enter_context(tc.tile_pool(name="xp", bufs=10))
            wpool = ctx.enter_context(tc.tile_pool(name="wp", bufs=3))
            apool = ctx.enter_context(tc.tile_pool(name="ap", bufs=3))
            opool = ctx.enter_context(tc.tile_pool(name="op", bufs=3))
            spool = ctx.enter_context(tc.tile_pool(name="sp", bufs=4))

            with tc.For_i(0, ROWS, 1) as iv:
                for (j0, m) in BLKS:
                    # load the 8 input rows for this output row / col block,
                    # q-expanded: xt[j, q, c] = xc[row, j0+j+q, c]
                    xts = []
                    for p in range(ROI):
                        xt = xpool.tile([128, ROI, CCH], f32, tag="xt")
                        base = xc[bass.ds(iv + p, 1), j0:j0 + m, :]
                        win = AP(base.tensor, base.offset,
                                 [[CCH, m], [CCH, ROI], [1, CCH]],
                                 dep_tracking_offset=base.dep_tracking_offset)
                        nc.sync.dma_start(xt[:m, :, :], win)
                        xts.append(xt)
                    # per-pixel weights for this row / col block
                    wtile = wpool.tile([128, TAPS], f32, tag="wt")
                    nc.sync.dma_start(
                        wtile[:m, :], wt[bass.ds(iv, 1), j0:j0 + m, :])

                    acc = apool.tile([128, CCH], f32, tag="acc")
                    for p in range(ROI):
                        for q in range(ROI):
                            t = p * ROI + q
                            if t == 0:
                                nc.vector.tensor_scalar(
                                    acc[:m, :], xts[p][:m, q, :],
                                    wtile[:m, 0:1], None,
                                    op0=mybir.AluOpType.mult)
                            else:
                                nc.vector.scalar_tensor_tensor(
                                    acc[:m, :], xts[p][:m, q, :],
                                    wtile[:m, t:t + 1], acc[:m, :],
                                    op0=mybir.AluOpType.mult,
                                    op1=mybir.AluOpType.add)

                    # normalize: out = acc[:, :C] / (acc[:, C] + 1e-6)
                    rec = spool.tile([128, 1], f32, tag="rec")
                    nc.vector.tensor_scalar_add(
                        rec[:m, :], acc[:m, C:C + 1], 1e-6)
                    nc.vector.reciprocal(rec[:m, :], rec[:m, :])
                    ot = opool.tile([128, C], f32, tag="ot")
                    nc.vector.tensor_scalar(
                        ot[:m, :], acc[:m, 0:C], rec[:m, 0:1], None,
                        op0=mybir.AluOpType.mult)
                    nc.sync.dma_start(out[bass.ds(iv, 1), j0:j0 + m, :],
                                      ot[:m, :])
    nc.compile()
    return nc


def _get_nc(rep=1):
    key = ("nc", rep)
    if key not in _CACHE:
        _CACHE[key] = _build_nc(rep)
    return _CACHE[key]


def _host_prep(x0, weights, cnts):
    """Build padded (x0 ++ cnts) array and per-core shards."""
    xc = np.zeros((H + ROI - 1, WPAD, CCH), np.float32)
    xc[PAD_LO:PAD_LO + H, PAD_LO:PAD_LO + W, :C] = x0
    xc[PAD_LO:PAD_LO + H, PAD_LO:PAD_LO + W, C] = cnts[:, :, 0]
    in_maps = []
    for k in range(NCORES):
        r0 = k * ROWS
        in_maps.append({
            "xc": np.ascontiguousarray(xc[r0:r0 + IN_ROWS]),
            "wt": np.ascontiguousarray(weights[r0:r0 + ROWS]),
        })
    return in_maps


def kernel(x0, weights, cnts):
    x0 = np.asarray(x0, np.float32)
    weights = np.asarray(weights, np.float32)
    cnts = np.asarray(cnts, np.float32)
    nc = _get_nc()
    in_maps = _host_prep(x0, weights, cnts)
    res = run_bass_kernel_spmd(nc, in_maps, core_ids=list(range(NCORES)))
    return np.concatenate([res.results[k]["out"] for k in range(NCORES)],
                          axis=0)


# revision 7
# speedup vs baseline: 6.8726x; 6.8726x over previous
"""Trainium2 Bass kernel for nn_CellAnnotator (per-pixel 8x8 locally-connected
weighted pooling with normalization), SPMD across 8 NeuronCores.

Contract: kernel(**inputs) takes FULL inputs (x0 [512,512,128] f32,
weights [512,512,64] f32, cnts [512,512,1] f32) and returns the FULL
output [512,512,128] f32.

Sharding: rows (H) split across 8 cores, 64 output rows each; each core's
input shard carries a 3+4-row halo (built host-side, zero-padded at the
image borders), so no device-to-device communication is needed.

Algorithm (banded matmul on the TensorEngine):
  out[i,j,c] = sum_{p,q} w[i,j,p*8+q] * x_pad[i+p, j+q, c], then divided by
  the same pooling applied to cnts (appended as channel 128 of x).
  For an output row i and a 57-pixel column block, the 64-tap sum is done as
  4 PSUM-accumulated matmuls, one per input-row pair: contraction dim
  K = 128 = (2 rows) x (64 input cols); lhsT is a banded [128, 57] weight
  tile (built host-side: weight w[i,j,t] sits at [64*g + (j-j0) + q, j-j0]);
  rhs is the bf16 input tile [128, 129] (128 channels + cnts).
"""

import numpy as np
import ml_dtypes
from contextlib import ExitStack

import concourse.bass as bass
import concourse.bacc as bacc
import concourse.mybir as mybir
import concourse.tile as tile
from concourse.ap import AP
from concourse.bass_utils import run_bass_kernel_spmd

BF16 = np.dtype(ml_dtypes.bfloat16)

# Problem constants (hardcoded per contract)
H, W, C = 512, 512, 128
ROI = 8
TAPS = ROI * ROI
PAD_LO, PAD_HI = 3, 4          # XLA SAME padding for even kernel
NCORES = 8
ROWS = H // NCORES             # 64 output rows per core
IN_ROWS = ROWS + ROI - 1       # 71 input rows (halo included)
WPAD = W + ROI                 # padded width: cols -3 .. 516 (520)
CCH = C + 1                    # x channels + cnts as channel 128

BLK = 57                       # output pixels per column block
NBLK = 9                       # 8*57 + 56 = 512
PPAIRS = 4                     # input-row pairs per output row
BFREE = PPAIRS * NBLK * BLK    # free size of one row's band line (2052)

_CACHE = {}


def _build_nc(rep=1):
    f32 = mybir.dt.float32
    bf = mybir.dt.bfloat16
    nc = bacc.Bacc("TRN2", target_bir_lowering=False, debug=False,
                   num_devices=NCORES)
    xcp = nc.dram_tensor("xcp", [IN_ROWS, WPAD, CCH], bf, kind="ExternalInput")
    bnd = nc.dram_tensor("bnd", [ROWS, 128, BFREE], bf, kind="ExternalInput")
    out = nc.dram_tensor("out", [ROWS, W, C], f32, kind="ExternalOutput")

    with tile.TileContext(nc) as tc:
        with ExitStack() as ctx:
            if rep > 1:
                ctx.enter_context(tc.For_i(0, rep, 1))
            xpool = ctx.enter_context(tc.tile_pool(name="xp", bufs=84))
            bpool = ctx.enter_context(tc.tile_pool(name="bp", bufs=3))
            ppool = ctx.enter_context(
                tc.tile_pool(name="pp", bufs=6, space="PSUM"))
            opool = ctx.enter_context(tc.tile_pool(name="op", bufs=4))
            spool = ctx.enter_context(tc.tile_pool(name="sp", bufs=4))

            xcache = {}

            def get_x(ri, b):
                """Input tile for row pair (ri, ri+1), col block b:
                [128 = 2x64 positions, 129 channels]."""
                key = (ri, b)
                if key not in xcache:
                    t = xpool.tile([128, CCH], bf, tag="xt")
                    nc.sync.dma_start(
                        t[:], xcp[ri:ri + 2, BLK * b:BLK * b + 64, :])
                    xcache[key] = t
                return xcache[key]

            for il in range(ROWS):
                btile = bpool.tile([128, BFREE], bf, tag="bt")
                nc.sync.dma_start(
                    btile[:],
                    AP(bnd, il * 128 * BFREE, [[BFREE, 128], [1, BFREE]]))
                for b in range(NBLK):
                    m = W - BLK * b if b == NBLK - 1 else BLK
                    psum = ppool.tile([BLK, CCH], f32, tag="ps")
                    for pp in range(PPAIRS):
                        xt = get_x(il + 2 * pp, b)
                        off = (pp * NBLK + b) * BLK
                        nc.tensor.matmul(
                            psum[:m, :], btile[:, off:off + m], xt[:],
                            start=(pp == 0), stop=(pp == PPAIRS - 1))
                    rec = spool.tile([BLK, 1], f32, tag="rec")
                    nc.vector.tensor_scalar_add(
                        rec[:m, :], psum[:m, C:C + 1], 1e-6)
                    nc.vector.reciprocal(rec[:m, :], rec[:m, :])
                    ot = opool.tile([BLK, C], f32, tag="ot")
                    nc.vector.tensor_scalar(
                        ot[:m, :], psum[:m, 0:C], rec[:m, 0:1], None,
                        op0=mybir.AluOpType.mult)
                    nc.sync.dma_start(
                        out[il, BLK * b:BLK * b + m, :], ot[:m, :])
    nc.compile()
    return nc


def _get_nc(rep=1):
    key = ("nc", rep)
    if key not in _CACHE:
        _CACHE[key] = _build_nc(rep)
    return _CACHE[key]


def _build_bands(weights):
    """bands[i, kpos, pp, b, jj] = w[i, 57b+jj, (2pp+g)*8 + d] at
    kpos = 64g + jj + d  (g = kpos//64, d in [0,8)); zero elsewhere."""
    wq = weights.reshape(H, W, ROI, ROI)          # [i, j, p, q]
    bands = np.zeros((H, 128, PPAIRS, NBLK, BLK), BF16)
    for b in range(NBLK):
        m = W - BLK * b if b == NBLK - 1 else BLK
        jv = np.arange(m)
        wb = wq[:, BLK * b:BLK * b + m]            # [H, m, 8, 8]
        for pp in range(PPAIRS):
            for g in range(2):
                p = 2 * pp + g
                for d in range(ROI):
                    bands[:, 64 * g + jv + d, pp, b, jv] = \
                        wb[:, jv, p, d].astype(BF16)
    return bands


def _host_prep(x0, weights, cnts):
    xcp = np.zeros((H + ROI - 1, WPAD, CCH), BF16)
    xcp[PAD_LO:PAD_LO + H, PAD_LO:PAD_LO + W, :C] = x0.astype(BF16)
    xcp[PAD_LO:PAD_LO + H, PAD_LO:PAD_LO + W, C] = cnts[:, :, 0].astype(BF16)
    bands = _build_bands(weights)
    in_maps = []
    for k in range(NCORES):
        r0 = k * ROWS
        in_maps.append({
            "xcp": np.ascontiguousarray(xcp[r0:r0 + IN_ROWS]),
            "bnd": np.ascontiguousarray(
                bands[r0:r0 + ROWS].reshape(ROWS, 128, BFREE)),
        })
    return in_maps


def kernel(x0, weights, cnts):
    x0 = np.asarray(x0, np.float32)
    weights = np.asarray(weights, np.float32)
    cnts = np.asarray(cnts, np.float32)
    nc = _get_nc()
    in_maps = _host_prep(x0, weights, cnts)
    res = run_bass_kernel_spmd(nc, in_maps, core_ids=list(range(NCORES)))
    return np.concatenate([res.results[k]["out"] for k in range(NCORES)],
                          axis=0)


# revision 8
# speedup vs baseline: 14.0184x; 2.0398x over previous
"""Trainium2 Bass kernel for nn_CellAnnotator (per-pixel 8x8 locally-connected
weighted pooling with normalization), SPMD across 8 NeuronCores.

Contract: kernel(**inputs) takes FULL inputs (x0 [512,512,128] f32,
weights [512,512,64] f32, cnts [512,512,1] f32) and returns the FULL
output [512,512,128] f32.

Sharding: rows (H) split across 8 cores, 64 output rows each; each core's
input shard carries a 3+4-row halo (built host-side, zero-padded at the
image borders), so no device-to-device communication is needed.

Algorithm (banded matmul on the TensorEngine):
  out[i,j,c] = sum_{p,q} w[i,j,p*8+q] * x_pad[i+p, j+q, c], normalized by
  the same pooling applied to cnts (appended as channel 128 of x).
  For an output row i and a 57-pixel column block, the 64-tap sum is done as
  4 PSUM-accumulated matmuls, one per input-row pair: contraction dim
  K = 128 = (2 rows) x (64 input cols); lhsT is a banded [128, 57] weight
  tile (built host-side: w[i,j,p*8+q] sits at [64*(p%2) + (j-j0) + q, j-j0]);
  rhs is the bf16 input tile [128, 129] (128 channels + cnts).

DMA batching: input tiles for one row-pair are loaded for all 9 column
blocks in 2 DMAs; band tiles come as one 525KB DMA per row; the output row
is staged in one SBUF tile (blocked pixel layout, de-blocked on host) and
stored with a single DMA.
"""

import numpy as np
import ml_dtypes
from contextlib import ExitStack

import concourse.bass as bass
import concourse.bacc as bacc
import concourse.mybir as mybir
import concourse.tile as tile
from concourse.ap import AP
from concourse.bass_utils import run_bass_kernel_spmd

BF16 = np.dtype(ml_dtypes.bfloat16)

# Problem constants (hardcoded per contract)
H, W, C = 512, 512, 128
ROI = 8
TAPS = ROI * ROI
PAD_LO, PAD_HI = 3, 4          # XLA SAME padding for even kernel
NCORES = 8
ROWS = H // NCORES             # 64 output rows per core
IN_ROWS = ROWS + ROI - 1       # 71 input rows (halo included)
WPAD = W + ROI                 # padded width: cols -3 .. 516 (520)
CCH = C + 1                    # x channels + cnts as channel 128

BLK = 57                       # output pixels per column block
NBLK = 9                       # 8*57 + 56 = 512
PPAIRS = 4                     # input-row pairs per output row
BFREE = PPAIRS * NBLK * BLK    # free size of one row's band line (2052)

_CACHE = {}


def _build_nc(rep=1):
    f32 = mybir.dt.float32
    bf = mybir.dt.bfloat16
    nc = bacc.Bacc("TRN2", target_bir_lowering=False, debug=False,
                   num_devices=NCORES)
    xcp = nc.dram_tensor("xcp", [IN_ROWS, WPAD, CCH], bf, kind="ExternalInput")
    bnd = nc.dram_tensor("bnd", [ROWS, 128, BFREE], bf, kind="ExternalInput")
    # blocked output layout: [row, jj, b, c]; de-blocked host-side
    out = nc.dram_tensor("out", [ROWS, BLK, NBLK, C], f32,
                         kind="ExternalOutput")

    with tile.TileContext(nc) as tc:
        with ExitStack() as ctx:
            if rep > 1:
                ctx.enter_context(tc.For_i(0, rep, 1))
            xpool = ctx.enter_context(tc.tile_pool(name="xp", bufs=12))
            bpool = ctx.enter_context(tc.tile_pool(name="bp", bufs=3))
            ppool = ctx.enter_context(
                tc.tile_pool(name="pp", bufs=6, space="PSUM"))
            opool = ctx.enter_context(tc.tile_pool(name="op", bufs=3))
            spool = ctx.enter_context(tc.tile_pool(name="sp", bufs=4))

            xcache = {}

            def get_x(ri):
                """Input tiles for row pair (ri, ri+1), all column blocks:
                [128 = 2x64 positions, 9 blocks, 129 channels]."""
                if ri not in xcache:
                    t = xpool.tile([128, NBLK, CCH], bf, tag="xt")
                    for g in range(2):
                        src = AP(xcp, (ri + g) * WPAD * CCH,
                                 [[CCH, 64], [BLK * CCH, NBLK], [1, CCH]])
                        nc.sync.dma_start(t[64 * g:64 * g + 64, :, :], src)
                    xcache[ri] = t
                return xcache[ri]

            for il in range(ROWS):
                btile = bpool.tile([128, PPAIRS, NBLK, BLK], bf, tag="bt")
                nc.sync.dma_start(
                    btile[:],
                    AP(bnd, il * 128 * BFREE, [[BFREE, 128], [1, BFREE]]))
                otile = opool.tile([BLK, NBLK, C], f32, tag="ot")
                for b in range(NBLK):
                    m = W - BLK * b if b == NBLK - 1 else BLK
                    psum = ppool.tile([BLK, CCH], f32, tag="ps")
                    for pp in range(PPAIRS):
                        xt = get_x(il + 2 * pp)
                        nc.tensor.matmul(
                            psum[:m, :], btile[:, pp, b, :m], xt[:, b, :],
                            start=(pp == 0), stop=(pp == PPAIRS - 1))
                    rec = spool.tile([BLK, 1], f32, tag="rec")
                    nc.vector.tensor_scalar_add(
                        rec[:m, :], psum[:m, C:C + 1], 1e-6)
                    nc.vector.reciprocal(rec[:m, :], rec[:m, :])
                    nc.vector.tensor_scalar(
                        otile[:m, b, :], psum[:m, 0:C], rec[:m, 0:1], None,
                        op0=mybir.AluOpType.mult)
                nc.sync.dma_start(out[il], otile[:])
    nc.compile()
    return nc


def _get_nc(rep=1):
    key = ("nc", rep)
    if key not in _CACHE:
        _CACHE[key] = _build_nc(rep)
    return _CACHE[key]


def _build_bands(weights):
    """bands[i, kpos, pp, b, jj] = w[i, 57b+jj, (2pp+g)*8 + d] at
    kpos = 64g + jj + d  (g = kpos//64, d in [0,8)); zero elsewhere."""
    wq = weights.reshape(H, W, ROI, ROI)          # [i, j, p, q]
    bands = np.zeros((H, 128, PPAIRS, NBLK, BLK), BF16)
    for b in range(NBLK):
        m = W - BLK * b if b == NBLK - 1 else BLK
        jv = np.arange(m)
        wb = wq[:, BLK * b:BLK * b + m]            # [H, m, 8, 8]
        for pp in range(PPAIRS):
            for g in range(2):
                p = 2 * pp + g
                for d in range(ROI):
                    bands[:, 64 * g + jv + d, pp, b, jv] = \
                        wb[:, jv, p, d].astype(BF16)
    return bands


def _host_prep(x0, weights, cnts):
    xcp = np.zeros((H + ROI - 1, WPAD, CCH), BF16)
    xcp[PAD_LO:PAD_LO + H, PAD_LO:PAD_LO + W, :C] = x0.astype(BF16)
    xcp[PAD_LO:PAD_LO + H, PAD_LO:PAD_LO + W, C] = cnts[:, :, 0].astype(BF16)
    bands = _build_bands(weights)
    in_maps = []
    for k in range(NCORES):
        r0 = k * ROWS
        in_maps.append({
            "xcp": np.ascontiguousarray(xcp[r0:r0 + IN_ROWS]),
            "bnd": np.ascontiguousarray(
                bands[r0:r0 + ROWS].reshape(ROWS, 128, BFREE)),
        })
    return in_maps


def _unblock(arr):
    """[ROWS, BLK, NBLK, C] blocked -> [ROWS, W, C]."""
    return arr.transpose(0, 2, 1, 3).reshape(ROWS, NBLK * BLK, C)[:, :W]


def kernel(x0, weights, cnts):
    x0 = np.asarray(x0, np.float32)
    weights = np.asarray(weights, np.float32)
    cnts = np.asarray(cnts, np.float32)
    nc = _get_nc()
    in_maps = _host_prep(x0, weights, cnts)
    res = run_bass_kernel_spmd(nc, in_maps, core_ids=list(range(NCORES)))
    return np.ascontiguousarray(np.concatenate(
        [_unblock(res.results[k]["out"]) for k in range(NCORES)], axis=0))
